# revision 40
# baseline (speedup 1.0000x reference)
"""DeepFM forward kernel for 8 Trainium2 NeuronCores.

Strategy (data-parallel, per the sharding hint): shard the batch of 2048
rows across 8 cores (256 rows each); replicate the embedding table, FM
linear weight, and MLP params.

v4 gather design: FOUR batched InstDMAGatherAnt gathers of 512 indices
each, one per (row-tile, field-quad), spread over the 4 SWDGE queues
(queue-parallel descriptor generation; measured ~4x faster than one
queue, and vastly faster than 16 serialized indirect DMAs). dma_gather
takes int16 indices, so the augmented table is PAIR-PACKED: table slot s
(256B stride, a hardware requirement on the source stride) holds rows 2s
and 2s+1 (17 f32 each); idx = gid >> 1 fits int16 (30000 < 32768), the
gather reads 136B (both rows; elem_size need not be 256B-aligned on HW,
verified), and a parity mask selects the right half on the DVE. This
keeps ONE descriptor per (row, field) slot with no zero-row hotspots.
Index layout (verified on HW): int16 block [16, n/16] with
blk[k%16, k//16] = idx of slot k, replicated to all 128 partitions;
slot k lands in out[k%128, k//128, :].

On-device per core: load mlp ucode library (needed by dma_gather; its
~10us load overlaps the input DMAs), 2 input DMAs, 4 dma_gathers,
parity-select + FM via DVE/ACT, MLP via PE matmuls in transposed-
activation form with both 128-row tiles batched into 256-wide matmuls,
1 output store.
"""

import numpy as np

import concourse.bass as bass
import concourse.bacc as bacc
import concourse.mybir as mybir
import concourse.tile as tile
from concourse import library_config
from concourse.bass_utils import run_bass_kernel_spmd
from concourse._compat import exact_div

N_CORES = 8
B = 2048
BC = B // N_CORES  # 256 rows per core
NT = BC // 128     # 2 tiles of 128 rows
F = 8              # fields
D = 16             # embed dim
NI = BC * F        # 2048 gather slots per core
NQ = NI // 8       # 256 indices per gather block (tile-0 round first)
EW = 128           # table slot stride in bf16 units (256B, holds a row pair)
RW = D + 1         # augmented row width (16 emb + 1 w_lin)
ES = 2 * RW        # gathered elem: both rows of the pair (34 bf16, 68B)
FIELD_DIMS = [50000, 5000, 2000, 1000, 1000, 500, 300, 200]
OFFSETS = np.concatenate([[0], np.cumsum(FIELD_DIMS)[:-1]]).astype(np.int64)
INPUT_DIM = int(np.sum(FIELD_DIMS))  # 60000
NRP = INPUT_DIM // 2                 # 30000 pair slots
H1, H2, H3 = 256, 128, 64

# f32 blob column layout ([128, BLOBW]): biases only
C_B1 = 0             # b1 [256] as 2 cols of 128
C_B2 = C_B1 + 2      # b2 [128]
C_B3 = C_B2 + 1      # b3 [64] in partitions 0..63
C_BLIN = C_B3 + 1    # b_lin broadcast to all partitions
BLOBW = C_BLIN + 1

# bf16 weight blob ([128, WBW]): MLP weights (single-pass PE matmuls),
# parity masks, and the transpose identity (per-core, because of the masks)
C_W1 = 0             # w1 [128, 256]
C_W2 = C_W1 + H1     # w2 chunks [128, 128] x2
C_W3 = C_W2 + H2 * 2  # w3 [128, 64]
C_WL = C_W3 + H3     # w_last [64] in partitions 0..63
C_BM16 = C_WL + 1    # parity mask expanded x16 [128, NT*F*16]
C_BM1 = C_BM16 + NT * F * D  # parity mask [128, NT*F]
C_BID = C_BM1 + NT * F  # 128x128 identity (PE transpose-by-matmul, bf16)
WBW = C_BID + 128

_CACHE = {}


def _raw_gather(gp, out_ap, in_ap, idxs_ap, num_idxs, elem_size, elem_step,
                queue_num, dtsize=4):
    """bass.dma_gather minus its 256B elem_size assert (transpose-only
    restriction; arbitrary elem_size verified correct on HW). The source
    STRIDE (elem_step) must still be a 256B multiple (ISA encoding)."""
    stride_bytes_256 = exact_div(elem_step * dtsize, 256)
    _in_ap = gp.lower_ap_dma(in_ap, for_custom_bir_dma=True)
    _idxs_ap = gp.lower_ap(idxs_ap)
    _out_ap = gp.lower_ap(out_ap)
    return gp.add_instruction(
        mybir.InstDMAGatherAnt(
            name=gp.bass.get_next_instruction_name(),
            ins=[*_in_ap, _idxs_ap,
                 gp.lower_val_access(gp.to_reg(num_idxs))],
            outs=[_out_ap],
            transpose=False,
            num_idxs=num_idxs,
            elem_size=elem_size,
            stride_bytes_256=stride_bytes_256,
            gen_mode=0,
            single_packet=True,
            queue_num=queue_num,
            sbuf_tokens_per_rank=0,
            sbuf_free_dim_per_rank=0,
            sbuf_free_dim_pad_per_rank=0,
            sbuf_byte_offset=0,
        )
    )


def build_program():
    """Build the single-core Bass/Tile program (SPMD: same NEFF on all cores)."""
    f32 = mybir.dt.float32
    i16 = mybir.dt.int16
    Alu = mybir.AluOpType
    Act = mybir.ActivationFunctionType

    # Bacc (not raw Bass): its lowering passes split/move multi-sem waits
    # (move_matmul_waits_to_ldweights, generate_event_semaphores) that the
    # TRN2 PE instruction encoding can't hold.
    bf16 = mybir.dt.bfloat16
    nc = bacc.Bacc(None, target_bir_lowering=False, num_swdge_queues=4,
                   enable_partition_id=False)
    tp = nc.dram_tensor("tp", [NRP, EW], bf16, kind="ExternalInput")
    blob = nc.dram_tensor("blob", [128, BLOBW], f32, kind="ExternalInput")
    wb = nc.dram_tensor("wb", [128, WBW], bf16, kind="ExternalInput")
    idx = nc.dram_tensor("idx", [128, 8 * (NQ // 16)], i16, kind="ExternalInput")
    y = nc.dram_tensor("y", [128, NT], f32, kind="ExternalOutput")

    # gpsimd ucode bank with InstDMAGatherAnt; ~10us async load, issued
    # before the tile-pool preamble so it overlaps the input DMAs.
    nc.gpsimd.load_library(library_config.mlp)

    with tile.TileContext(nc) as tc:
        with (
            tc.tile_pool(name="sb", bufs=2) as sp,
            tc.tile_pool(name="cst", bufs=1) as cp,
            tc.tile_pool(name="ps", bufs=1, space="PSUM") as pp,
        ):
            idx_t = cp.tile([128, 8 * (NQ // 16)], i16)
            nc.sync.dma_start(out=idx_t[:], in_=idx[:])
            blob_t = cp.tile([128, BLOBW], f32)
            nc.sync.dma_start(out=blob_t[:], in_=blob[:])
            wb_t = cp.tile([128, WBW], bf16)
            nc.sync.dma_start(out=wb_t[:], in_=wb[:])

            # g[p, j, 0:34] = bf16 pair containing the row of (row-tile
            # j//8, row 128*(j//8)+p, field j%8); gather block b = 4*i + q
            # covers row-tile i's fields [2q, 2q+2) on queue q — all of
            # tile 0's blocks run as the first queue-parallel round so its
            # select/FM/MLP pipeline starts while tile 1 still gathers.
            g = cp.tile([128, (NI // 128) * ES], bf16)
            g3 = g[:].rearrange("p (j e) -> p j e", e=ES)
            for b in range(8):
                i, q = b // 4, b % 4
                _raw_gather(
                    nc.gpsimd,
                    g3[:, i * F + q * 2:i * F + (q + 1) * 2, :],
                    tp[:, 0:ES],
                    idx_t[:, b * (NQ // 16):(b + 1) * (NQ // 16)],
                    NQ, ES, EW, queue_num=q, dtsize=2,
                )

            # parity select: hc = lo + m*(hi - lo) (per-slot row parity),
            # all in bf16 (2x DVE rate), done per row-tile so tile 0's
            # FM/MLP starts as soon as its two gather blocks land.
            hc = sp.tile([128, NT * F * D], bf16)
            de = sp.tile([128, NT * F * D], bf16)
            wl = sp.tile([128, NT * F], bf16)
            dw = sp.tile([128, NT * F], bf16)
            lin = sp.tile([128, NT], f32)
            y_sb = cp.tile([128, NT], f32)
            hT = sp.tile([128, NT * 128], bf16)
            fm2 = sp.tile([128, NT], f32)
            # emit tile 1's chain FIRST: the scheduler hoists ready work,
            # so tile 0 (whose gathers land a round earlier) fills the
            # engine queues ahead of tile 1's transfer-gated ops instead
            # of stalling behind them.
            for i in (1, 0):
                jl, jh = i * F, (i + 1) * F
                nc.vector.tensor_sub(
                    out=de[:, jl * D:jh * D].rearrange("p (j d) -> p j d", d=D),
                    in0=g3[:, jl:jh, RW:RW + D], in1=g3[:, jl:jh, 0:D],
                )
                nc.vector.tensor_mul(
                    out=de[:, jl * D:jh * D], in0=de[:, jl * D:jh * D],
                    in1=wb_t[:, C_BM16 + jl * D:C_BM16 + jh * D],
                )
                nc.vector.tensor_add(
                    out=hc[:, jl * D:jh * D].rearrange("p (j d) -> p j d", d=D),
                    in0=de[:, jl * D:jh * D].rearrange("p (j d) -> p j d", d=D),
                    in1=g3[:, jl:jh, 0:D],
                )
                nc.vector.tensor_sub(
                    out=dw[:, jl:jh].rearrange("p (j o) -> p j o", o=1),
                    in0=g3[:, jl:jh, RW + D:RW + D + 1],
                    in1=g3[:, jl:jh, D:D + 1],
                )
                nc.vector.tensor_mul(
                    out=dw[:, jl:jh], in0=dw[:, jl:jh],
                    in1=wb_t[:, C_BM1 + jl:C_BM1 + jh],
                )
                nc.vector.tensor_add(
                    out=wl[:, jl:jh].rearrange("p (j o) -> p j o", o=1),
                    in0=dw[:, jl:jh].rearrange("p (j o) -> p j o", o=1),
                    in1=g3[:, jl:jh, D:D + 1],
                )
                nc.vector.reduce_sum(
                    out=lin[:, i:i + 1], in_=wl[:, jl:jh],
                    axis=mybir.AxisListType.X,
                )

                hci = hc[:, i * F * D:(i + 1) * F * D]
                hc3 = hci.rearrange("p (f d) -> p f d", f=F)

                # FM field-sum tree (DVE, bf16)
                s4 = sp.tile([128, 4 * D], bf16)
                nc.vector.tensor_add(
                    out=s4[:].rearrange("p (f d) -> p f d", f=4),
                    in0=hc3[:, 0:4, :], in1=hc3[:, 4:8, :],
                )
                s43 = s4[:].rearrange("p (f d) -> p f d", f=4)
                s2 = sp.tile([128, 2 * D], bf16)
                nc.vector.tensor_add(
                    out=s2[:].rearrange("p (f d) -> p f d", f=2),
                    in0=s43[:, 0:2, :], in1=s43[:, 2:4, :],
                )
                s23 = s2[:].rearrange("p (f d) -> p f d", f=2)
                s1 = sp.tile([128, D], bf16)
                nc.vector.tensor_add(
                    out=s1[:].rearrange("p (f d) -> p f d", f=1),
                    in0=s23[:, 0:1, :], in1=s23[:, 1:2, :],
                )

                # transpose this tile's activations (bf16 single-pass PE)
                hT_p = pp.tile([128, 128], f32)
                nc.tensor.matmul(
                    out=hT_p[:], lhsT=hci, rhs=wb_t[:, C_BID:C_BID + 128],
                    start=True, stop=True,
                )
                hTi = hT[:, i * 128:(i + 1) * 128]
                nc.vector.tensor_copy(out=hTi, in_=hT_p[:])

                # per-tile MLP (128-wide matmuls) so tile 0's chain runs
                # while tile 1 is still gathering
                a1 = sp.tile([128, 2 * 128], bf16)
                for c in range(2):
                    p1 = pp.tile([128, 128], f32)
                    nc.tensor.matmul(
                        out=p1[:],
                        lhsT=wb_t[:, C_W1 + c * 128:C_W1 + (c + 1) * 128],
                        rhs=hTi, start=True, stop=True,
                    )
                    nc.scalar.activation(
                        out=a1[:, c * 128:(c + 1) * 128], in_=p1[:],
                        func=Act.Relu,
                        bias=blob_t[:, C_B1 + c:C_B1 + c + 1], scale=1.0,
                    )
                p2 = pp.tile([128, 128], f32)
                nc.tensor.matmul(
                    out=p2[:], lhsT=wb_t[:, C_W2:C_W2 + 128],
                    rhs=a1[:, 0:128], start=True, stop=False,
                )
                nc.tensor.matmul(
                    out=p2[:], lhsT=wb_t[:, C_W2 + 128:C_W2 + 256],
                    rhs=a1[:, 128:256], start=False, stop=True,
                )
                a2 = sp.tile([128, 128], bf16)
                nc.scalar.activation(
                    out=a2[:], in_=p2[:], func=Act.Relu,
                    bias=blob_t[:, C_B2:C_B2 + 1], scale=1.0,
                )
                p3 = pp.tile([64, 128], f32)
                nc.tensor.matmul(
                    out=p3[:], lhsT=wb_t[:, C_W3:C_W3 + H3], rhs=a2[:],
                    start=True, stop=True,
                )
                a3 = sp.tile([64, 128], bf16)
                nc.scalar.activation(
                    out=a3[:], in_=p3[:], func=Act.Relu,
                    bias=blob_t[0:64, C_B3:C_B3 + 1], scale=1.0,
                )
                py = pp.tile([128, 1], f32)
                nc.tensor.matmul(
                    out=py[:], lhsT=a3[:],
                    rhs=wb_t[0:64, C_WL:C_WL + 1], start=True, stop=True,
                )

                # FM squares AFTER the MLP relus in the ACT queue so they
                # don't block the MLP chain (tensor_tensor_reduce crashes
                # the HW device; ACT Square with accum_out instead)
                sq = sp.tile([128, F * D], bf16)
                r2 = sp.tile([128, 1], f32)
                nc.scalar.activation(
                    out=sq[:], in_=hci, func=Act.Square, accum_out=r2[:],
                )
                ss = sp.tile([128, D], bf16)
                r1 = sp.tile([128, 1], f32)
                nc.scalar.activation(
                    out=ss[:], in_=s1[:], func=Act.Square, accum_out=r1[:],
                )
                nc.vector.tensor_sub(out=fm2[:, i:i + 1], in0=r1[:], in1=r2[:])

                # y_i = 0.5*fm2 + lin + b_lin + y_dnn
                t1 = sp.tile([128, 1], f32)
                nc.vector.scalar_tensor_tensor(
                    out=t1[:], in0=fm2[:, i:i + 1], scalar=0.5,
                    in1=lin[:, i:i + 1], op0=Alu.mult, op1=Alu.add,
                )
                t2 = sp.tile([128, 1], f32)
                nc.vector.tensor_add(
                    out=t2[:], in0=py[:], in1=blob_t[:, C_BLIN:C_BLIN + 1],
                )
                nc.vector.tensor_add(
                    out=y_sb[:, i:i + 1], in0=t1[:], in1=t2[:],
                )

            nc.sync.dma_start(out=y[:], in_=y_sb[:])
    nc.finalize()  # runs Bacc's lowering passes; the PJRT exec path requires it
    return nc


def prepare_inputs(x, emb_table, w_lin, b_lin, w1, b1, w2, b2, w3, b3, w_last):
    x = np.asarray(x)
    xoff = (x.astype(np.int64) + OFFSETS[None, :]).astype(np.int32)  # [2048, 8]

    # pair-packed augmented table: slot s = rows 2s, 2s+1 (17 bf16 each,
    # padded to a 256B slot stride)
    import ml_dtypes
    aug = np.zeros((INPUT_DIM, RW), np.float32)
    aug[:, :D] = np.asarray(emb_table, np.float32)
    aug[:, D] = np.asarray(w_lin, np.float32)
    tp = np.zeros((NRP, EW), ml_dtypes.bfloat16)
    tp[:, 0:RW] = aug[0::2]
    tp[:, RW:2 * RW] = aug[1::2]

    # per-core gather indices, 8 blocks of NQ=256 (row-tile i, field-pair
    # q): block b = 4*i + q covers local slots u = (f-2q)*128 + p; int16
    # block [16, NQ/16] with blk[u%16, u//16], replicated to all 128
    # partitions (the 8 gpsimd cores each read their own copy).
    g = xoff.reshape(N_CORES, NT, 128, F)            # [c, i, p, f]
    gid = np.ascontiguousarray(g.transpose(0, 1, 3, 2)).reshape(
        N_CORES, NT, 4, NQ)                          # [c, i, q, u]
    ip = (gid >> 1).astype(np.int16)
    idxc = np.zeros((N_CORES, 128, 8 * (NQ // 16)), np.int16)
    for i in range(NT):
        for q in range(4):
            b = 4 * i + q
            blk = ip[:, i, q].reshape(
                N_CORES, NQ // 16, 16).transpose(0, 2, 1)
            idxc[:, :, b * (NQ // 16):(b + 1) * (NQ // 16)] = np.tile(
                blk, (1, 8, 1))
    # parity mask per slot: m[c, p, j] = gid & 1 for slot k = j*128 + p
    par = (gid & 1).astype(np.float32).reshape(
        N_CORES, NT, F, 128).transpose(0, 3, 1, 2).reshape(
        N_CORES, 128, NT * F)                        # [c, p, j]

    blob = np.zeros((128, BLOBW), np.float32)
    b1 = np.asarray(b1, np.float32)
    blob[:, C_B1] = b1[0:128]
    blob[:, C_B1 + 1] = b1[128:256]
    blob[:, C_B2] = np.asarray(b2, np.float32)
    blob[0:H3, C_B3] = np.asarray(b3, np.float32)
    blob[:, C_BLIN] = np.float32(np.asarray(b_lin))

    wb = np.zeros((N_CORES, 128, WBW), ml_dtypes.bfloat16)
    wb[:, :, C_W1:C_W1 + H1] = np.asarray(w1, np.float32)
    w2 = np.asarray(w2, np.float32)
    wb[:, :, C_W2:C_W2 + H2] = w2[0:128, :]
    wb[:, :, C_W2 + H2:C_W2 + 2 * H2] = w2[128:256, :]
    wb[:, :, C_W3:C_W3 + H3] = np.asarray(w3, np.float32)
    wb[:, 0:H3, C_WL] = np.asarray(w_last, np.float32)[:, 0]
    wb[:, :, C_BM16:C_BM16 + NT * F * D] = np.repeat(par, D, axis=2)
    wb[:, :, C_BM1:C_BM1 + NT * F] = par
    wb[:, :, C_BID:C_BID + 128] = np.eye(128, dtype=np.float32)
    return tp, blob, wb, idxc


def kernel(**inputs):
    tp, blob, wb, idxc = prepare_inputs(**inputs)
    if "nc" not in _CACHE:
        _CACHE["nc"] = build_program()
    nc = _CACHE["nc"]
    in_maps = [
        {"tp": tp, "blob": blob, "wb": wb[c], "idx": idxc[c]}
        for c in range(N_CORES)
    ]
    res = run_bass_kernel_spmd(nc, in_maps, list(range(N_CORES))).results
    # y[c*256 + i*128 + p] = res[c]["y"][p, i]
    out = np.concatenate([res[c]["y"].T.reshape(BC) for c in range(N_CORES)])
    return out.astype(np.float32)


if __name__ == "__main__":
    rng = np.random.default_rng(0)
    demo = {
        "x": np.stack([rng.integers(0, FIELD_DIMS[f], 2048) for f in range(F)], 1).astype(np.int64),
        "emb_table": rng.standard_normal((INPUT_DIM, D), np.float32) * 0.01,
        "w_lin": rng.random(INPUT_DIM, np.float32),
        "b_lin": np.float32(0.0),
        "w1": rng.standard_normal((F * D, H1), np.float32) * 0.1,
        "b1": np.zeros(H1, np.float32),
        "w2": rng.standard_normal((H1, H2), np.float32) * 0.1,
        "b2": np.zeros(H2, np.float32),
        "w3": rng.standard_normal((H2, H3), np.float32) * 0.1,
        "b3": np.zeros(H3, np.float32),
        "w_last": rng.standard_normal((H3, 1), np.float32) * 0.1,
    }
    print(kernel(**demo)[:8])


# revision 41
# speedup vs baseline: 1.0103x; 1.0103x over previous
"""DeepFM forward kernel for 8 Trainium2 NeuronCores.

Strategy (data-parallel, per the sharding hint): shard the batch of 2048
rows across 8 cores (256 rows each); replicate the embedding table, FM
linear weight, and MLP params.

v4 gather design: FOUR batched InstDMAGatherAnt gathers of 512 indices
each, one per (row-tile, field-quad), spread over the 4 SWDGE queues
(queue-parallel descriptor generation; measured ~4x faster than one
queue, and vastly faster than 16 serialized indirect DMAs). dma_gather
takes int16 indices, so the augmented table is PAIR-PACKED: table slot s
(256B stride, a hardware requirement on the source stride) holds rows 2s
and 2s+1 (17 f32 each); idx = gid >> 1 fits int16 (30000 < 32768), the
gather reads 136B (both rows; elem_size need not be 256B-aligned on HW,
verified), and a parity mask selects the right half on the DVE. This
keeps ONE descriptor per (row, field) slot with no zero-row hotspots.
Index layout (verified on HW): int16 block [16, n/16] with
blk[k%16, k//16] = idx of slot k, replicated to all 128 partitions;
slot k lands in out[k%128, k//128, :].

On-device per core: load mlp ucode library (needed by dma_gather; its
~10us load overlaps the input DMAs), 2 input DMAs, 4 dma_gathers,
parity-select + FM via DVE/ACT, MLP via PE matmuls in transposed-
activation form with both 128-row tiles batched into 256-wide matmuls,
1 output store.
"""

import numpy as np

import concourse.bass as bass
import concourse.bacc as bacc
import concourse.mybir as mybir
import concourse.tile as tile
from concourse import library_config
from concourse.bass_utils import run_bass_kernel_spmd
from concourse._compat import exact_div

N_CORES = 8
B = 2048
BC = B // N_CORES  # 256 rows per core
NT = BC // 128     # 2 tiles of 128 rows
F = 8              # fields
D = 16             # embed dim
NI = BC * F        # 2048 gather slots per core
NQ = NI // 8       # 256 indices per gather block (tile-0 round first)
EW = 128           # table slot stride in bf16 units (256B, holds a row pair)
RW = D + 1         # augmented row width (16 emb + 1 w_lin)
ES = 2 * RW        # gathered elem: both rows of the pair (34 bf16, 68B)
FIELD_DIMS = [50000, 5000, 2000, 1000, 1000, 500, 300, 200]
OFFSETS = np.concatenate([[0], np.cumsum(FIELD_DIMS)[:-1]]).astype(np.int64)
INPUT_DIM = int(np.sum(FIELD_DIMS))  # 60000
NRP = INPUT_DIM // 2                 # 30000 pair slots
H1, H2, H3 = 256, 128, 64

# f32 blob column layout ([128, BLOBW]): biases only
C_B1 = 0             # b1 [256] as 2 cols of 128
C_B2 = C_B1 + 2      # b2 [128]
C_B3 = C_B2 + 1      # b3 [64] in partitions 0..63
C_BLIN = C_B3 + 1    # b_lin broadcast to all partitions
BLOBW = C_BLIN + 1

# bf16 weight blob ([128, WBW]): MLP weights (single-pass PE matmuls),
# parity masks, and the transpose identity (per-core, because of the masks)
C_W1 = 0             # w1 [128, 256]
C_W2 = C_W1 + H1     # w2 chunks [128, 128] x2
C_W3 = C_W2 + H2 * 2  # w3 [128, 64]
C_WL = C_W3 + H3     # w_last [64] in partitions 0..63
C_BM16 = C_WL + 1    # parity mask expanded x16 [128, NT*F*16]
C_BM1 = C_BM16 + NT * F * D  # parity mask [128, NT*F]
C_BID = C_BM1 + NT * F  # 128x128 identity (PE transpose-by-matmul, bf16)
WBW = C_BID + 128

_CACHE = {}


def _raw_gather(gp, out_ap, in_ap, idxs_ap, num_idxs, elem_size, elem_step,
                queue_num, dtsize=4):
    """bass.dma_gather minus its 256B elem_size assert (transpose-only
    restriction; arbitrary elem_size verified correct on HW). The source
    STRIDE (elem_step) must still be a 256B multiple (ISA encoding)."""
    stride_bytes_256 = exact_div(elem_step * dtsize, 256)
    _in_ap = gp.lower_ap_dma(in_ap, for_custom_bir_dma=True)
    _idxs_ap = gp.lower_ap(idxs_ap)
    _out_ap = gp.lower_ap(out_ap)
    return gp.add_instruction(
        mybir.InstDMAGatherAnt(
            name=gp.bass.get_next_instruction_name(),
            ins=[*_in_ap, _idxs_ap,
                 gp.lower_val_access(gp.to_reg(num_idxs))],
            outs=[_out_ap],
            transpose=False,
            num_idxs=num_idxs,
            elem_size=elem_size,
            stride_bytes_256=stride_bytes_256,
            gen_mode=0,
            single_packet=True,
            queue_num=queue_num,
            sbuf_tokens_per_rank=0,
            sbuf_free_dim_per_rank=0,
            sbuf_free_dim_pad_per_rank=0,
            sbuf_byte_offset=0,
        )
    )


def build_program():
    """Build the single-core Bass/Tile program (SPMD: same NEFF on all cores)."""
    f32 = mybir.dt.float32
    i16 = mybir.dt.int16
    Alu = mybir.AluOpType
    Act = mybir.ActivationFunctionType

    # Bacc (not raw Bass): its lowering passes split/move multi-sem waits
    # (move_matmul_waits_to_ldweights, generate_event_semaphores) that the
    # TRN2 PE instruction encoding can't hold.
    bf16 = mybir.dt.bfloat16
    nc = bacc.Bacc(None, target_bir_lowering=False, num_swdge_queues=4,
                   enable_partition_id=False)
    tp = nc.dram_tensor("tp", [NRP, EW], bf16, kind="ExternalInput")
    blob = nc.dram_tensor("blob", [128, BLOBW], f32, kind="ExternalInput")
    wb = nc.dram_tensor("wb", [128, WBW], bf16, kind="ExternalInput")
    idx = nc.dram_tensor("idx", [128, 8 * (NQ // 16)], i16, kind="ExternalInput")
    y = nc.dram_tensor("y", [128, NT], f32, kind="ExternalOutput")

    # gpsimd ucode bank with InstDMAGatherAnt; ~10us async load, issued
    # before the tile-pool preamble so it overlaps the input DMAs.
    nc.gpsimd.load_library(library_config.mlp)

    with tile.TileContext(nc) as tc:
        with (
            tc.tile_pool(name="sb", bufs=2) as sp,
            tc.tile_pool(name="cst", bufs=1) as cp,
            tc.tile_pool(name="ps", bufs=1, space="PSUM") as pp,
        ):
            idx_t = cp.tile([128, 8 * (NQ // 16)], i16)
            nc.sync.dma_start(out=idx_t[:], in_=idx[:])
            blob_t = cp.tile([128, BLOBW], f32)
            nc.sync.dma_start(out=blob_t[:], in_=blob[:])
            wb_t = cp.tile([128, WBW], bf16)
            nc.sync.dma_start(out=wb_t[:], in_=wb[:])

            # g[p, j, 0:34] = bf16 pair containing the row of (row-tile
            # j//8, row 128*(j//8)+p, field j%8); gather block b = 4*i + q
            # covers row-tile i's fields [2q, 2q+2) on queue q — all of
            # tile 0's blocks run as the first queue-parallel round so its
            # select/FM/MLP pipeline starts while tile 1 still gathers.
            g = cp.tile([128, (NI // 128) * ES], bf16)
            g3 = g[:].rearrange("p (j e) -> p j e", e=ES)
            for b in range(8):
                i, q = b // 4, b % 4
                _raw_gather(
                    nc.gpsimd,
                    g3[:, i * F + q * 2:i * F + (q + 1) * 2, :],
                    tp[:, 0:ES],
                    idx_t[:, b * (NQ // 16):(b + 1) * (NQ // 16)],
                    NQ, ES, EW, queue_num=q, dtsize=2,
                )

            # parity select: hc = lo + m*(hi - lo) (per-slot row parity),
            # all in bf16 (2x DVE rate), done per row-tile so tile 0's
            # FM/MLP starts as soon as its two gather blocks land.
            hc = sp.tile([128, NT * F * D], bf16)
            de = sp.tile([128, NT * F * D], bf16)
            wl = sp.tile([128, NT * F], bf16)
            dw = sp.tile([128, NT * F], bf16)
            lin = sp.tile([128, NT], f32)
            y_sb = cp.tile([128, NT], f32)
            hT = sp.tile([128, NT * 128], bf16)
            fm2 = sp.tile([128, NT], f32)
            for i in range(NT):
                jl, jh = i * F, (i + 1) * F
                nc.vector.tensor_sub(
                    out=de[:, jl * D:jh * D].rearrange("p (j d) -> p j d", d=D),
                    in0=g3[:, jl:jh, RW:RW + D], in1=g3[:, jl:jh, 0:D],
                )
                nc.vector.tensor_mul(
                    out=de[:, jl * D:jh * D], in0=de[:, jl * D:jh * D],
                    in1=wb_t[:, C_BM16 + jl * D:C_BM16 + jh * D],
                )
                nc.vector.tensor_add(
                    out=hc[:, jl * D:jh * D].rearrange("p (j d) -> p j d", d=D),
                    in0=de[:, jl * D:jh * D].rearrange("p (j d) -> p j d", d=D),
                    in1=g3[:, jl:jh, 0:D],
                )
                nc.vector.tensor_sub(
                    out=dw[:, jl:jh].rearrange("p (j o) -> p j o", o=1),
                    in0=g3[:, jl:jh, RW + D:RW + D + 1],
                    in1=g3[:, jl:jh, D:D + 1],
                )
                nc.vector.tensor_mul(
                    out=dw[:, jl:jh], in0=dw[:, jl:jh],
                    in1=wb_t[:, C_BM1 + jl:C_BM1 + jh],
                )
                nc.vector.tensor_add(
                    out=wl[:, jl:jh].rearrange("p (j o) -> p j o", o=1),
                    in0=dw[:, jl:jh].rearrange("p (j o) -> p j o", o=1),
                    in1=g3[:, jl:jh, D:D + 1],
                )
                nc.vector.reduce_sum(
                    out=lin[:, i:i + 1], in_=wl[:, jl:jh],
                    axis=mybir.AxisListType.X,
                )

                hci = hc[:, i * F * D:(i + 1) * F * D]
                hc3 = hci.rearrange("p (f d) -> p f d", f=F)

                # FM field-sum tree (DVE, bf16)
                s4 = sp.tile([128, 4 * D], bf16)
                nc.vector.tensor_add(
                    out=s4[:].rearrange("p (f d) -> p f d", f=4),
                    in0=hc3[:, 0:4, :], in1=hc3[:, 4:8, :],
                )
                s43 = s4[:].rearrange("p (f d) -> p f d", f=4)
                s2 = sp.tile([128, 2 * D], bf16)
                nc.vector.tensor_add(
                    out=s2[:].rearrange("p (f d) -> p f d", f=2),
                    in0=s43[:, 0:2, :], in1=s43[:, 2:4, :],
                )
                s23 = s2[:].rearrange("p (f d) -> p f d", f=2)
                s1 = sp.tile([128, D], bf16)
                nc.vector.tensor_add(
                    out=s1[:].rearrange("p (f d) -> p f d", f=1),
                    in0=s23[:, 0:1, :], in1=s23[:, 1:2, :],
                )

                # transpose this tile's activations (bf16 single-pass PE)
                hT_p = pp.tile([128, 128], f32)
                nc.tensor.matmul(
                    out=hT_p[:], lhsT=hci, rhs=wb_t[:, C_BID:C_BID + 128],
                    start=True, stop=True,
                )
                hTi = hT[:, i * 128:(i + 1) * 128]
                nc.vector.tensor_copy(out=hTi, in_=hT_p[:])

                # per-tile MLP (128-wide matmuls) so tile 0's chain runs
                # while tile 1 is still gathering
                a1 = sp.tile([128, 2 * 128], bf16)
                for c in range(2):
                    p1 = pp.tile([128, 128], f32)
                    nc.tensor.matmul(
                        out=p1[:],
                        lhsT=wb_t[:, C_W1 + c * 128:C_W1 + (c + 1) * 128],
                        rhs=hTi, start=True, stop=True,
                    )
                    nc.scalar.activation(
                        out=a1[:, c * 128:(c + 1) * 128], in_=p1[:],
                        func=Act.Relu,
                        bias=blob_t[:, C_B1 + c:C_B1 + c + 1], scale=1.0,
                    )
                p2 = pp.tile([128, 128], f32)
                nc.tensor.matmul(
                    out=p2[:], lhsT=wb_t[:, C_W2:C_W2 + 128],
                    rhs=a1[:, 0:128], start=True, stop=False,
                )
                nc.tensor.matmul(
                    out=p2[:], lhsT=wb_t[:, C_W2 + 128:C_W2 + 256],
                    rhs=a1[:, 128:256], start=False, stop=True,
                )
                a2 = sp.tile([128, 128], bf16)
                nc.scalar.activation(
                    out=a2[:], in_=p2[:], func=Act.Relu,
                    bias=blob_t[:, C_B2:C_B2 + 1], scale=1.0,
                )
                p3 = pp.tile([64, 128], f32)
                nc.tensor.matmul(
                    out=p3[:], lhsT=wb_t[:, C_W3:C_W3 + H3], rhs=a2[:],
                    start=True, stop=True,
                )
                a3 = sp.tile([64, 128], bf16)
                nc.scalar.activation(
                    out=a3[:], in_=p3[:], func=Act.Relu,
                    bias=blob_t[0:64, C_B3:C_B3 + 1], scale=1.0,
                )
                py = pp.tile([128, 1], f32)
                nc.tensor.matmul(
                    out=py[:], lhsT=a3[:],
                    rhs=wb_t[0:64, C_WL:C_WL + 1], start=True, stop=True,
                )

                # FM squares AFTER the MLP relus in the ACT queue so they
                # don't block the MLP chain (tensor_tensor_reduce crashes
                # the HW device; ACT Square with accum_out instead)
                sq = sp.tile([128, F * D], bf16)
                r2 = sp.tile([128, 1], f32)
                nc.scalar.activation(
                    out=sq[:], in_=hci, func=Act.Square, accum_out=r2[:],
                )
                ss = sp.tile([128, D], bf16)
                r1 = sp.tile([128, 1], f32)
                nc.scalar.activation(
                    out=ss[:], in_=s1[:], func=Act.Square, accum_out=r1[:],
                )
                nc.vector.tensor_sub(out=fm2[:, i:i + 1], in0=r1[:], in1=r2[:])

                # y_i = 0.5*fm2 + lin + b_lin + y_dnn
                t1 = sp.tile([128, 1], f32)
                nc.vector.scalar_tensor_tensor(
                    out=t1[:], in0=fm2[:, i:i + 1], scalar=0.5,
                    in1=lin[:, i:i + 1], op0=Alu.mult, op1=Alu.add,
                )
                t2 = sp.tile([128, 1], f32)
                nc.vector.tensor_add(
                    out=t2[:], in0=py[:], in1=blob_t[:, C_BLIN:C_BLIN + 1],
                )
                nc.vector.tensor_add(
                    out=y_sb[:, i:i + 1], in0=t1[:], in1=t2[:],
                )

            nc.sync.dma_start(out=y[:], in_=y_sb[:])
    nc.finalize()  # runs Bacc's lowering passes; the PJRT exec path requires it
    return nc


def prepare_inputs(x, emb_table, w_lin, b_lin, w1, b1, w2, b2, w3, b3, w_last):
    x = np.asarray(x)
    xoff = (x.astype(np.int64) + OFFSETS[None, :]).astype(np.int32)  # [2048, 8]

    # pair-packed augmented table: slot s = rows 2s, 2s+1 (17 bf16 each,
    # padded to a 256B slot stride)
    import ml_dtypes
    aug = np.zeros((INPUT_DIM, RW), np.float32)
    aug[:, :D] = np.asarray(emb_table, np.float32)
    aug[:, D] = np.asarray(w_lin, np.float32)
    tp = np.zeros((NRP, EW), ml_dtypes.bfloat16)
    tp[:, 0:RW] = aug[0::2]
    tp[:, RW:2 * RW] = aug[1::2]

    # per-core gather indices, 8 blocks of NQ=256 (row-tile i, field-pair
    # q): block b = 4*i + q covers local slots u = (f-2q)*128 + p; int16
    # block [16, NQ/16] with blk[u%16, u//16], replicated to all 128
    # partitions (the 8 gpsimd cores each read their own copy).
    g = xoff.reshape(N_CORES, NT, 128, F)            # [c, i, p, f]
    gid = np.ascontiguousarray(g.transpose(0, 1, 3, 2)).reshape(
        N_CORES, NT, 4, NQ)                          # [c, i, q, u]
    ip = (gid >> 1).astype(np.int16)
    idxc = np.zeros((N_CORES, 128, 8 * (NQ // 16)), np.int16)
    for i in range(NT):
        for q in range(4):
            b = 4 * i + q
            blk = ip[:, i, q].reshape(
                N_CORES, NQ // 16, 16).transpose(0, 2, 1)
            idxc[:, :, b * (NQ // 16):(b + 1) * (NQ // 16)] = np.tile(
                blk, (1, 8, 1))
    # parity mask per slot: m[c, p, j] = gid & 1 for slot k = j*128 + p
    par = (gid & 1).astype(np.float32).reshape(
        N_CORES, NT, F, 128).transpose(0, 3, 1, 2).reshape(
        N_CORES, 128, NT * F)                        # [c, p, j]

    blob = np.zeros((128, BLOBW), np.float32)
    b1 = np.asarray(b1, np.float32)
    blob[:, C_B1] = b1[0:128]
    blob[:, C_B1 + 1] = b1[128:256]
    blob[:, C_B2] = np.asarray(b2, np.float32)
    blob[0:H3, C_B3] = np.asarray(b3, np.float32)
    blob[:, C_BLIN] = np.float32(np.asarray(b_lin))

    wb = np.zeros((N_CORES, 128, WBW), ml_dtypes.bfloat16)
    wb[:, :, C_W1:C_W1 + H1] = np.asarray(w1, np.float32)
    w2 = np.asarray(w2, np.float32)
    wb[:, :, C_W2:C_W2 + H2] = w2[0:128, :]
    wb[:, :, C_W2 + H2:C_W2 + 2 * H2] = w2[128:256, :]
    wb[:, :, C_W3:C_W3 + H3] = np.asarray(w3, np.float32)
    wb[:, 0:H3, C_WL] = np.asarray(w_last, np.float32)[:, 0]
    wb[:, :, C_BM16:C_BM16 + NT * F * D] = np.repeat(par, D, axis=2)
    wb[:, :, C_BM1:C_BM1 + NT * F] = par
    wb[:, :, C_BID:C_BID + 128] = np.eye(128, dtype=np.float32)
    return tp, blob, wb, idxc


def kernel(**inputs):
    tp, blob, wb, idxc = prepare_inputs(**inputs)
    if "nc" not in _CACHE:
        _CACHE["nc"] = build_program()
    nc = _CACHE["nc"]
    in_maps = [
        {"tp": tp, "blob": blob, "wb": wb[c], "idx": idxc[c]}
        for c in range(N_CORES)
    ]
    res = run_bass_kernel_spmd(nc, in_maps, list(range(N_CORES))).results
    # y[c*256 + i*128 + p] = res[c]["y"][p, i]
    out = np.concatenate([res[c]["y"].T.reshape(BC) for c in range(N_CORES)])
    return out.astype(np.float32)


if __name__ == "__main__":
    rng = np.random.default_rng(0)
    demo = {
        "x": np.stack([rng.integers(0, FIELD_DIMS[f], 2048) for f in range(F)], 1).astype(np.int64),
        "emb_table": rng.standard_normal((INPUT_DIM, D), np.float32) * 0.01,
        "w_lin": rng.random(INPUT_DIM, np.float32),
        "b_lin": np.float32(0.0),
        "w1": rng.standard_normal((F * D, H1), np.float32) * 0.1,
        "b1": np.zeros(H1, np.float32),
        "w2": rng.standard_normal((H1, H2), np.float32) * 0.1,
        "b2": np.zeros(H2, np.float32),
        "w3": rng.standard_normal((H2, H3), np.float32) * 0.1,
        "b3": np.zeros(H3, np.float32),
        "w_last": rng.standard_normal((H3, 1), np.float32) * 0.1,
    }
    print(kernel(**demo)[:8])


# revision 46
# speedup vs baseline: 1.0199x; 1.0095x over previous
"""DeepFM forward kernel for 8 Trainium2 NeuronCores.

Strategy (data-parallel, per the sharding hint): shard the batch of 2048
rows across 8 cores (256 rows each); replicate the embedding table, FM
linear weight, and MLP params.

v4 gather design: FOUR batched InstDMAGatherAnt gathers of 512 indices
each, one per (row-tile, field-quad), spread over the 4 SWDGE queues
(queue-parallel descriptor generation; measured ~4x faster than one
queue, and vastly faster than 16 serialized indirect DMAs). dma_gather
takes int16 indices, so the augmented table is PAIR-PACKED: table slot s
(256B stride, a hardware requirement on the source stride) holds rows 2s
and 2s+1 (17 f32 each); idx = gid >> 1 fits int16 (30000 < 32768), the
gather reads 136B (both rows; elem_size need not be 256B-aligned on HW,
verified), and a parity mask selects the right half on the DVE. This
keeps ONE descriptor per (row, field) slot with no zero-row hotspots.
Index layout (verified on HW): int16 block [16, n/16] with
blk[k%16, k//16] = idx of slot k, replicated to all 128 partitions;
slot k lands in out[k%128, k//128, :].

On-device per core: load mlp ucode library (needed by dma_gather; its
~10us load overlaps the input DMAs), 2 input DMAs, 4 dma_gathers,
parity-select + FM via DVE/ACT, MLP via PE matmuls in transposed-
activation form with both 128-row tiles batched into 256-wide matmuls,
1 output store.
"""

import numpy as np

import concourse.bass as bass
import concourse.bacc as bacc
import concourse.mybir as mybir
import concourse.tile as tile
from concourse import library_config
from concourse.bass_utils import run_bass_kernel_spmd
from concourse._compat import exact_div

N_CORES = 8
B = 2048
BC = B // N_CORES  # 256 rows per core
NT = BC // 128     # 2 tiles of 128 rows
F = 8              # fields
D = 16             # embed dim
NI = BC * F        # 2048 gather slots per core
NQ = NI // 4       # 512 indices per gather block (one queue-parallel round)
EW = 128           # table slot stride in bf16 units (256B, holds a row pair)
RW = D + 1         # augmented row width (16 emb + 1 w_lin)
ES = 2 * RW        # gathered elem: both rows of the pair (34 bf16, 68B)
FIELD_DIMS = [50000, 5000, 2000, 1000, 1000, 500, 300, 200]
OFFSETS = np.concatenate([[0], np.cumsum(FIELD_DIMS)[:-1]]).astype(np.int64)
INPUT_DIM = int(np.sum(FIELD_DIMS))  # 60000
NRP = INPUT_DIM // 2                 # 30000 pair slots
H1, H2, H3 = 256, 128, 64

# f32 blob column layout ([128, BLOBW]): biases only
C_B1 = 0             # b1 [256] as 2 cols of 128
C_B2 = C_B1 + 2      # b2 [128]
C_B3 = C_B2 + 1      # b3 [64] in partitions 0..63
C_BLIN = C_B3 + 1    # b_lin broadcast to all partitions
BLOBW = C_BLIN + 1

# bf16 weight blob ([128, WBW]): MLP weights (single-pass PE matmuls),
# parity masks, and the transpose identity (per-core, because of the masks)
C_W1 = 0             # w1 [128, 256]
C_W2 = C_W1 + H1     # w2 chunks [128, 128] x2
C_W3 = C_W2 + H2 * 2  # w3 [128, 64]
C_WL = C_W3 + H3     # w_last [64] in partitions 0..63
C_BM16 = C_WL + 1    # parity mask expanded x16 [128, NT*F*16]
C_BM1 = C_BM16 + NT * F * D  # parity mask [128, NT*F]
C_BID = C_BM1 + NT * F  # 128x128 identity (PE transpose-by-matmul, bf16)
WBW = C_BID + 128

_CACHE = {}


def _raw_gather(gp, out_ap, in_ap, idxs_ap, num_idxs, elem_size, elem_step,
                queue_num, dtsize=4):
    """bass.dma_gather minus its 256B elem_size assert (transpose-only
    restriction; arbitrary elem_size verified correct on HW). The source
    STRIDE (elem_step) must still be a 256B multiple (ISA encoding)."""
    stride_bytes_256 = exact_div(elem_step * dtsize, 256)
    _in_ap = gp.lower_ap_dma(in_ap, for_custom_bir_dma=True)
    _idxs_ap = gp.lower_ap(idxs_ap)
    _out_ap = gp.lower_ap(out_ap)
    return gp.add_instruction(
        mybir.InstDMAGatherAnt(
            name=gp.bass.get_next_instruction_name(),
            ins=[*_in_ap, _idxs_ap,
                 gp.lower_val_access(gp.to_reg(num_idxs))],
            outs=[_out_ap],
            transpose=False,
            num_idxs=num_idxs,
            elem_size=elem_size,
            stride_bytes_256=stride_bytes_256,
            gen_mode=0,
            single_packet=True,
            queue_num=queue_num,
            sbuf_tokens_per_rank=0,
            sbuf_free_dim_per_rank=0,
            sbuf_free_dim_pad_per_rank=0,
            sbuf_byte_offset=0,
        )
    )


def build_program():
    """Build the single-core Bass/Tile program (SPMD: same NEFF on all cores)."""
    f32 = mybir.dt.float32
    i16 = mybir.dt.int16
    Alu = mybir.AluOpType
    Act = mybir.ActivationFunctionType

    # Bacc (not raw Bass): its lowering passes split/move multi-sem waits
    # (move_matmul_waits_to_ldweights, generate_event_semaphores) that the
    # TRN2 PE instruction encoding can't hold.
    bf16 = mybir.dt.bfloat16
    nc = bacc.Bacc(None, target_bir_lowering=False, num_swdge_queues=4,
                   enable_partition_id=False)
    tp = nc.dram_tensor("tp", [NRP, EW], bf16, kind="ExternalInput")
    blob = nc.dram_tensor("blob", [128, BLOBW], f32, kind="ExternalInput")
    wb = nc.dram_tensor("wb", [128, WBW], bf16, kind="ExternalInput")
    idx = nc.dram_tensor("idx", [128, 4 * (NQ // 16)], i16, kind="ExternalInput")
    y = nc.dram_tensor("y", [128, NT], f32, kind="ExternalOutput")

    # gpsimd ucode bank with InstDMAGatherAnt; ~10us async load, issued
    # before the tile-pool preamble so it overlaps the input DMAs.
    nc.gpsimd.load_library(library_config.mlp)

    with tile.TileContext(nc) as tc:
        with (
            tc.tile_pool(name="sb", bufs=2) as sp,
            tc.tile_pool(name="cst", bufs=1) as cp,
            tc.tile_pool(name="ps", bufs=1, space="PSUM") as pp,
        ):
            idx_t = cp.tile([128, 4 * (NQ // 16)], i16)
            nc.sync.dma_start(out=idx_t[:], in_=idx[:])
            blob_t = cp.tile([128, BLOBW], f32)
            nc.sync.dma_start(out=blob_t[:], in_=blob[:])
            wb_t = cp.tile([128, WBW], bf16)
            nc.sync.dma_start(out=wb_t[:], in_=wb[:])

            # g[p, j, 0:34] = bf16 pair containing the row of (row-tile
            # j//8, row 128*(j//8)+p, field j%8); gather block b = 2*i + s
            # covers row-tile i's fields [4s, 4s+4) on queue b — ONE
            # queue-parallel round, so both tiles' transfers land together
            # and no engine queue stalls head-of-line on late tile-1 data.
            g = cp.tile([128, (NI // 128) * ES], bf16)
            g3 = g[:].rearrange("p (j e) -> p j e", e=ES)
            for b in range(4):
                i, s = b // 2, b % 2
                _raw_gather(
                    nc.gpsimd,
                    g3[:, i * F + s * 4:i * F + (s + 1) * 4, :],
                    tp[:, 0:ES],
                    idx_t[:, b * (NQ // 16):(b + 1) * (NQ // 16)],
                    NQ, ES, EW, queue_num=b, dtsize=2,
                )

            # parity select: hc = lo + m*(hi - lo) (per-slot row parity),
            # all in bf16 (2x DVE rate), done per row-tile so tile 0's
            # FM/MLP starts as soon as its two gather blocks land.
            hc = sp.tile([128, NT * F * D], bf16)
            de = sp.tile([128, NT * F * D], bf16)
            wl = sp.tile([128, NT * F], bf16)
            dw = sp.tile([128, NT * F], bf16)
            lin = sp.tile([128, NT], f32)
            y_sb = cp.tile([128, NT], f32)
            hT = sp.tile([128, NT * 128], bf16)
            fm2 = sp.tile([128, NT], f32)
            for i in range(NT):
                jl, jh = i * F, (i + 1) * F
                nc.vector.tensor_sub(
                    out=de[:, jl * D:jh * D].rearrange("p (j d) -> p j d", d=D),
                    in0=g3[:, jl:jh, RW:RW + D], in1=g3[:, jl:jh, 0:D],
                )
                nc.vector.tensor_mul(
                    out=de[:, jl * D:jh * D], in0=de[:, jl * D:jh * D],
                    in1=wb_t[:, C_BM16 + jl * D:C_BM16 + jh * D],
                )
                nc.vector.tensor_add(
                    out=hc[:, jl * D:jh * D].rearrange("p (j d) -> p j d", d=D),
                    in0=de[:, jl * D:jh * D].rearrange("p (j d) -> p j d", d=D),
                    in1=g3[:, jl:jh, 0:D],
                )
                nc.vector.tensor_sub(
                    out=dw[:, jl:jh].rearrange("p (j o) -> p j o", o=1),
                    in0=g3[:, jl:jh, RW + D:RW + D + 1],
                    in1=g3[:, jl:jh, D:D + 1],
                )
                nc.vector.tensor_mul(
                    out=dw[:, jl:jh], in0=dw[:, jl:jh],
                    in1=wb_t[:, C_BM1 + jl:C_BM1 + jh],
                )
                nc.vector.tensor_add(
                    out=wl[:, jl:jh].rearrange("p (j o) -> p j o", o=1),
                    in0=dw[:, jl:jh].rearrange("p (j o) -> p j o", o=1),
                    in1=g3[:, jl:jh, D:D + 1],
                )
                nc.vector.reduce_sum(
                    out=lin[:, i:i + 1], in_=wl[:, jl:jh],
                    axis=mybir.AxisListType.X,
                )

                hci = hc[:, i * F * D:(i + 1) * F * D]
                hc3 = hci.rearrange("p (f d) -> p f d", f=F)

                # FM field-sum tree (DVE, bf16)
                s4 = sp.tile([128, 4 * D], bf16)
                nc.vector.tensor_add(
                    out=s4[:].rearrange("p (f d) -> p f d", f=4),
                    in0=hc3[:, 0:4, :], in1=hc3[:, 4:8, :],
                )
                s43 = s4[:].rearrange("p (f d) -> p f d", f=4)
                s2 = sp.tile([128, 2 * D], bf16)
                nc.vector.tensor_add(
                    out=s2[:].rearrange("p (f d) -> p f d", f=2),
                    in0=s43[:, 0:2, :], in1=s43[:, 2:4, :],
                )
                s23 = s2[:].rearrange("p (f d) -> p f d", f=2)
                s1 = sp.tile([128, D], bf16)
                nc.vector.tensor_add(
                    out=s1[:].rearrange("p (f d) -> p f d", f=1),
                    in0=s23[:, 0:1, :], in1=s23[:, 1:2, :],
                )

                # transpose this tile's activations (bf16 single-pass PE)
                hT_p = pp.tile([128, 128], f32)
                nc.tensor.matmul(
                    out=hT_p[:], lhsT=hci, rhs=wb_t[:, C_BID:C_BID + 128],
                    start=True, stop=True,
                )
                hTi = hT[:, i * 128:(i + 1) * 128]
                nc.vector.tensor_copy(out=hTi, in_=hT_p[:])

                # per-tile MLP (128-wide matmuls) so tile 0's chain runs
                # while tile 1 is still gathering
                a1 = sp.tile([128, 2 * 128], bf16)
                for c in range(2):
                    p1 = pp.tile([128, 128], f32)
                    nc.tensor.matmul(
                        out=p1[:],
                        lhsT=wb_t[:, C_W1 + c * 128:C_W1 + (c + 1) * 128],
                        rhs=hTi, start=True, stop=True,
                    )
                    nc.scalar.activation(
                        out=a1[:, c * 128:(c + 1) * 128], in_=p1[:],
                        func=Act.Relu,
                        bias=blob_t[:, C_B1 + c:C_B1 + c + 1], scale=1.0,
                    )
                p2 = pp.tile([128, 128], f32)
                nc.tensor.matmul(
                    out=p2[:], lhsT=wb_t[:, C_W2:C_W2 + 128],
                    rhs=a1[:, 0:128], start=True, stop=False,
                )
                nc.tensor.matmul(
                    out=p2[:], lhsT=wb_t[:, C_W2 + 128:C_W2 + 256],
                    rhs=a1[:, 128:256], start=False, stop=True,
                )
                a2 = sp.tile([128, 128], bf16)
                nc.scalar.activation(
                    out=a2[:], in_=p2[:], func=Act.Relu,
                    bias=blob_t[:, C_B2:C_B2 + 1], scale=1.0,
                )
                p3 = pp.tile([64, 128], f32)
                nc.tensor.matmul(
                    out=p3[:], lhsT=wb_t[:, C_W3:C_W3 + H3], rhs=a2[:],
                    start=True, stop=True,
                )
                a3 = sp.tile([64, 128], bf16)
                nc.scalar.activation(
                    out=a3[:], in_=p3[:], func=Act.Relu,
                    bias=blob_t[0:64, C_B3:C_B3 + 1], scale=1.0,
                )
                py = pp.tile([128, 1], f32)
                nc.tensor.matmul(
                    out=py[:], lhsT=a3[:],
                    rhs=wb_t[0:64, C_WL:C_WL + 1], start=True, stop=True,
                )

                # FM squares AFTER the MLP relus in the ACT queue so they
                # don't block the MLP chain (tensor_tensor_reduce crashes
                # the HW device; ACT Square with accum_out instead)
                sq = sp.tile([128, F * D], bf16)
                r2 = sp.tile([128, 1], f32)
                nc.scalar.activation(
                    out=sq[:], in_=hci, func=Act.Square, accum_out=r2[:],
                )
                ss = sp.tile([128, D], bf16)
                r1 = sp.tile([128, 1], f32)
                nc.scalar.activation(
                    out=ss[:], in_=s1[:], func=Act.Square, accum_out=r1[:],
                )
                nc.vector.tensor_sub(out=fm2[:, i:i + 1], in0=r1[:], in1=r2[:])

                # y_i = 0.5*fm2 + lin + b_lin + y_dnn
                t1 = sp.tile([128, 1], f32)
                nc.vector.scalar_tensor_tensor(
                    out=t1[:], in0=fm2[:, i:i + 1], scalar=0.5,
                    in1=lin[:, i:i + 1], op0=Alu.mult, op1=Alu.add,
                )
                t2 = sp.tile([128, 1], f32)
                nc.vector.tensor_add(
                    out=t2[:], in0=py[:], in1=blob_t[:, C_BLIN:C_BLIN + 1],
                )
                nc.vector.tensor_add(
                    out=y_sb[:, i:i + 1], in0=t1[:], in1=t2[:],
                )

            nc.sync.dma_start(out=y[:], in_=y_sb[:])
    nc.finalize()  # runs Bacc's lowering passes; the PJRT exec path requires it
    return nc


def prepare_inputs(x, emb_table, w_lin, b_lin, w1, b1, w2, b2, w3, b3, w_last):
    x = np.asarray(x)
    xoff = (x.astype(np.int64) + OFFSETS[None, :]).astype(np.int32)  # [2048, 8]

    # pair-packed augmented table: slot s = rows 2s, 2s+1 (17 bf16 each,
    # padded to a 256B slot stride)
    import ml_dtypes
    aug = np.zeros((INPUT_DIM, RW), np.float32)
    aug[:, :D] = np.asarray(emb_table, np.float32)
    aug[:, D] = np.asarray(w_lin, np.float32)
    tp = np.zeros((NRP, EW), ml_dtypes.bfloat16)
    tp[:, 0:RW] = aug[0::2]
    tp[:, RW:2 * RW] = aug[1::2]

    # per-core gather indices, 4 blocks of NQ=512 (row-tile i, field-quad
    # s): block b = 2*i + s covers local slots u = (f-4s)*128 + p; int16
    # block [16, NQ/16] with blk[u%16, u//16], replicated to all 128
    # partitions (the 8 gpsimd cores each read their own copy).
    g = xoff.reshape(N_CORES, NT, 128, F)            # [c, i, p, f]
    gid = np.ascontiguousarray(g.transpose(0, 1, 3, 2)).reshape(
        N_CORES, NT, 2, NQ)                          # [c, i, s, u]
    ip = (gid >> 1).astype(np.int16)
    idxc = np.zeros((N_CORES, 128, 4 * (NQ // 16)), np.int16)
    for i in range(NT):
        for s in range(2):
            b = 2 * i + s
            blk = ip[:, i, s].reshape(
                N_CORES, NQ // 16, 16).transpose(0, 2, 1)
            idxc[:, :, b * (NQ // 16):(b + 1) * (NQ // 16)] = np.tile(
                blk, (1, 8, 1))
    # parity mask per slot: m[c, p, j] = gid & 1 for slot k = j*128 + p
    par = (gid & 1).astype(np.float32).reshape(
        N_CORES, NT, F, 128).transpose(0, 3, 1, 2).reshape(
        N_CORES, 128, NT * F)                        # [c, p, j]

    blob = np.zeros((128, BLOBW), np.float32)
    b1 = np.asarray(b1, np.float32)
    blob[:, C_B1] = b1[0:128]
    blob[:, C_B1 + 1] = b1[128:256]
    blob[:, C_B2] = np.asarray(b2, np.float32)
    blob[0:H3, C_B3] = np.asarray(b3, np.float32)
    blob[:, C_BLIN] = np.float32(np.asarray(b_lin))

    wb = np.zeros((N_CORES, 128, WBW), ml_dtypes.bfloat16)
    wb[:, :, C_W1:C_W1 + H1] = np.asarray(w1, np.float32)
    w2 = np.asarray(w2, np.float32)
    wb[:, :, C_W2:C_W2 + H2] = w2[0:128, :]
    wb[:, :, C_W2 + H2:C_W2 + 2 * H2] = w2[128:256, :]
    wb[:, :, C_W3:C_W3 + H3] = np.asarray(w3, np.float32)
    wb[:, 0:H3, C_WL] = np.asarray(w_last, np.float32)[:, 0]
    wb[:, :, C_BM16:C_BM16 + NT * F * D] = np.repeat(par, D, axis=2)
    wb[:, :, C_BM1:C_BM1 + NT * F] = par
    wb[:, :, C_BID:C_BID + 128] = np.eye(128, dtype=np.float32)
    return tp, blob, wb, idxc


def kernel(**inputs):
    tp, blob, wb, idxc = prepare_inputs(**inputs)
    if "nc" not in _CACHE:
        _CACHE["nc"] = build_program()
    nc = _CACHE["nc"]
    in_maps = [
        {"tp": tp, "blob": blob, "wb": wb[c], "idx": idxc[c]}
        for c in range(N_CORES)
    ]
    res = run_bass_kernel_spmd(nc, in_maps, list(range(N_CORES))).results
    # y[c*256 + i*128 + p] = res[c]["y"][p, i]
    out = np.concatenate([res[c]["y"].T.reshape(BC) for c in range(N_CORES)])
    return out.astype(np.float32)


if __name__ == "__main__":
    rng = np.random.default_rng(0)
    demo = {
        "x": np.stack([rng.integers(0, FIELD_DIMS[f], 2048) for f in range(F)], 1).astype(np.int64),
        "emb_table": rng.standard_normal((INPUT_DIM, D), np.float32) * 0.01,
        "w_lin": rng.random(INPUT_DIM, np.float32),
        "b_lin": np.float32(0.0),
        "w1": rng.standard_normal((F * D, H1), np.float32) * 0.1,
        "b1": np.zeros(H1, np.float32),
        "w2": rng.standard_normal((H1, H2), np.float32) * 0.1,
        "b2": np.zeros(H2, np.float32),
        "w3": rng.standard_normal((H2, H3), np.float32) * 0.1,
        "b3": np.zeros(H3, np.float32),
        "w_last": rng.standard_normal((H3, 1), np.float32) * 0.1,
    }
    print(kernel(**demo)[:8])
